# revision 2
# baseline (speedup 1.0000x reference)
r"""Trainium2 Bass kernel for DeepRBFNetwork distances, v2 (chol + engine split).

Math: distances[b, k] = || features[b] @ A[k].T + b[k] ||_2
  = sqrt( ||f L_k||^2 + f . (2 A_k^T b_k) + ||b_k||^2 ),  L_k = chol(A_k^T A_k)

L_k is lower-triangular: the (d-block 0, e-block 1) 256x256 quadrant is zero,
so each (bt, k) pair needs one full DoubleRow matmul (d 256..511, e 0..511,
ap 512) plus one quarter matmul (d 0..255, e 0..255, ap 256) - 25% less PE
streaming than the plain A^T form.  All operands fp8e4m3, scaled by
SL = 2^7.5 (L) so that Q_raw = sum t_raw^2 = Q * 2^15 matches the affine
bank's scale (c2 * 2^15, g * 2^15); one ACT Sqrt(scale=2^-15) descales all.

Per-pair epilogue is split across engines to run under the PE roofline:
  - ACT route: one fused activation(Square, accum_out=q_col) per pair.
  - DVE route: one bn_stats per 2-bank duo; sum x^2 = M2_e + 256 m_e^2 +
    M2_o + 256 m_o^2 recovered with 3 batched DVE ops per super-group.
Assembly: S = aff_psum + q (DVE stt, psum read), sqrt+store batched per 4 bt.

Sharding: K padded 100->104, 13 classes per core x 8 cores, full batch per
core (output gathered on host along K).
"""

import os
import sys
import numpy as np
import ml_dtypes

GPOFF = int(os.environ.get("BASS_KERNEL_GPOFF", "1"))

import concourse.bacc as bacc
import concourse.bass as bass
import concourse.mybir as mybir
import concourse.tile as tile
from concourse.bass_utils import run_bass_kernel_spmd

B, K, D = 4096, 100, 512
NCORES = 8
KPAD = 104
KSH = KPAD // NCORES   # 13
NBT = B // 128         # 32

BF16 = mybir.dt.bfloat16
FP8 = mybir.dt.float8e4
F32 = mybir.dt.float32
AF = mybir.ActivationFunctionType
ALU = mybir.AluOpType
DRMODE = mybir.MatmulPerfMode.DoubleRow

SL2_LOG2 = 15                  # scale of Q_raw in psum (= SL^2)
SC2_LOG2 = 12                  # scale of the affine bank (c2, g) - fp8 range
SL = float(2.0 ** (SL2_LOG2 / 2.0))   # L pre-scale

LAST_EXEC_TIME_NS = None
LAST_RESULTS = None


def build_nc(n_bt: int = NBT):
    nc = bacc.Bacc(
        "TRN2", target_bir_lowering=False, debug=False, num_devices=NCORES
    )
    ftd = nc.dram_tensor("ftd", [128, 16384], FP8, kind="ExternalInput")
    ltd = nc.dram_tensor("ltd", [KSH, 128, 1536], FP8, kind="ExternalInput")
    c2d = nc.dram_tensor("c2d", [128, 2 * 2 * KSH], FP8, kind="ExternalInput")
    gtd = nc.dram_tensor("gtd", [128, KSH], F32, kind="ExternalInput")
    out = nc.dram_tensor("dist", [n_bt * 128, KSH], F32, kind="ExternalOutput")

    SQB = 16   # batch tiles per sqrt/output batch (big: avoids ACT
               # Square<->Sqrt activation-table thrash)
    SGW = 14   # pairs per super-group
    NDVE = 8   # leading pairs of each super-group routed to DVE (bn_stats)
    LAG = 6
    AFF_RATE = 3

    with tile.TileContext(nc) as tc:
        with (
            tc.tile_pool(name="const", bufs=1) as cpool,
            tc.tile_pool(name="dpsum", bufs=4, space="PSUM") as dpool,
            tc.tile_pool(name="cpsum", bufs=3, space="PSUM") as capool,
            tc.tile_pool(name="apsum", bufs=1, space="PSUM") as apool,
            tc.tile_pool(name="stp", bufs=3) as stpool,
            tc.tile_pool(name="sqs", bufs=2) as sqpool,
            tc.tile_pool(name="outp", bufs=3) as opool,
        ):
            # ---- DMAs (main operands first, affine operands arrive late)
            ft_t = cpool.tile([128, 2, 2, B], FP8, tag="ft")
            ftdv = ftd.rearrange("p (a b c) -> p a b c", a=2, b=2)
            lt_t = cpool.tile([128, KSH, 2, 768], FP8, tag="lt")
            ltf = lt_t.rearrange("p k a e -> p k (a e)")
            # staged lead-in: the first split super-group (bt 0..3, k=0)
            # can start as soon as the first two transfers land
            c2_t = cpool.tile([128, 2, 2, KSH], FP8, tag="c2")
            gt_t = cpool.tile([128, KSH], F32, tag="gt")
            nc.gpsimd.dma_start(ltf[:, 0], ltd[0])
            nc.sync.dma_start(ft_t[:, :, :, :512], ftdv[:, :, :, :512])
            # c2/gt are tiny; land them early so the affine matmuls never
            # stall the in-order PE queue
            nc.gpsimd.dma_start(
                c2_t.rearrange("p a b k -> p (a b k)")[:], c2d[:]
            )
            nc.gpsimd.dma_start(gt_t[:], gtd[:])
            nc.sync.dma_start(ft_t[:, :, :, 512:1792], ftdv[:, :, :, 512:1792])
            for k in range(1, KSH):
                nc.gpsimd.dma_start(ltf[:, k], ltd[k])
            nc.sync.dma_start(ft_t[:, :, :, 1792:], ftdv[:, :, :, 1792:])

            qbig = cpool.tile([128, n_bt, KSH], F32, tag="qbig")
            qflat = qbig.rearrange("p b k -> p (b k)")
            affb = apool.tile([128, 512], F32, tag="affb")

            def emit_affine(bt):
                aff = affb[:, bt * KSH:(bt + 1) * KSH]
                for pr in range(2):
                    nc.tensor.matmul(
                        aff,
                        ft_t[:, pr, :, bt * 128:(bt + 1) * 128],
                        c2_t[:, pr],
                        start=(pr == 0),
                        stop=(pr == 1),
                        perf_mode=DRMODE,
                    )

            def emit_pair(pg, j, bt, k):
                # full block: d 256..511 x e 0..511 (resets the bank)
                nc.tensor.matmul(
                    pg[:],
                    ft_t[:, 1, :, bt * 128:(bt + 1) * 128],
                    lt_t[:, k, :, :512],
                    start=True, stop=False,
                    perf_mode=DRMODE,
                )
                # quarter block: d 0..255 x e 0..255 (accumulates)
                nc.tensor.matmul(
                    pg[:, :256],
                    ft_t[:, 0, :, bt * 128:(bt + 1) * 128],
                    lt_t[:, k, :, 512:768],
                    start=False, stop=True,
                    perf_mode=DRMODE,
                    skip_group_check=True,
                )

            # ---- super-groups: phase 1 k-major (DMA overlap), then flat
            h1 = 14
            sgs = []
            for k in range(KSH):
                sgs.append([(bt, k) for bt in range(h1)])
            rest = [(bt, k) for bt in range(h1, n_bt) for k in range(KSH)]
            for i in range(0, len(rest), SGW):
                sgs.append(rest[i:i + SGW])
            # split the very first super-group so the pipeline primes early
            if len(sgs[0]) == SGW:
                sgs[0:1] = [sgs[0][0:4], sgs[0][4:8], sgs[0][8:14]]

            aff_done = 0
            done_upto = 0
            cols_done = [0] * n_bt
            s4_tile = [None]

            def qdest(sg, lo, hi):
                bt0, k0 = sg[0]
                if all(k == k0 for _, k in sg):       # phase 1: fixed k
                    return qbig[:, bt0 + lo:bt0 + hi, k0]
                c0 = bt0 * KSH + k0
                return qflat[:, c0 + lo:c0 + hi]

            def qcol(bt, k):
                c = bt * KSH + k
                return qflat[:, c:c + 1]

            sq_batches = [16, 12, 4]
            sq_bounds = []
            acc = 0
            for w in sq_batches:
                acc += w
                sq_bounds.append(acc)   # [16, 28, 32]

            def emit_assembly(bt):
                bi = next(i for i, e in enumerate(sq_bounds) if bt < e)
                b0 = sq_bounds[bi - 1] if bi else 0
                j = bt - b0
                if j == 0:
                    s4_tile[0] = opool.tile(
                        [128, SQB, KSH], F32, tag="s4", name="s4"
                    )
                s4 = s4_tile[0]
                nc.vector.scalar_tensor_tensor(
                    s4[:, j, :],
                    affb[:, bt * KSH:(bt + 1) * KSH],
                    2.0 ** (SL2_LOG2 - SC2_LOG2),
                    qbig[:, bt, :],
                    ALU.mult, ALU.add,
                )
                nc.gpsimd.tensor_tensor(
                    s4[:, j, :], s4[:, j, :], gt_t[:], op=ALU.add
                )
                if bt == sq_bounds[bi] - 1:
                    nn = j + 1
                    d4 = opool.tile([128, SQB, KSH], F32, tag="d4")
                    nc.scalar.activation(
                        d4[:, :nn, :], s4[:, :nn, :], AF.Sqrt,
                        scale=2.0 ** (-SL2_LOG2),
                    )
                    for jj in range(nn):
                        bx = bt - nn + 1 + jj
                        nc.sync.dma_start(
                            out[bx * 128:(bx + 1) * 128, :], d4[:, jj, :]
                        )

            n_p1 = KSH + 2   # phase-1 super-group count (after split)
            for si, sg in enumerate(sgs):
                npairs = len(sg)
                if si < n_p1:
                    nd = NDVE
                elif si < n_p1 + 7:
                    nd = NDVE
                elif si >= len(sgs) - 3:
                    nd = NDVE - 2
                else:
                    nd = NDVE - 1
                ndve = min(nd, max(0, npairs - 2)) if npairs > 2 else 0
                nact = npairs - ndve
                # interleave the fill order so both consumers start early;
                # routing is by pair INDEX (DVE block first) so the q
                # destinations stay contiguous for the batched reduce
                order = []
                di, ai = 0, ndve
                while di < ndve or ai < npairs:
                    if di < ndve:
                        order.append(di); di += 1
                    if ai < npairs:
                        order.append(ai); ai += 1
                st = stpool.tile([128, NDVE, 6], F32, tag="st")
                for pi in order:
                    bt, k = sg[pi]
                    if pi < ndve:
                        pg = dpool.tile([128, 512], F32, tag="pg", name="pg")
                        emit_pair(pg, None, bt, k)
                        nc.vector.bn_stats(st[:, pi, :], pg[:])
                    else:
                        pg = capool.tile([128, 512], F32, tag="pa", name="pa")
                        emit_pair(pg, None, bt, k)
                        sq = sqpool.tile([128, 512], BF16, tag="sq")
                        nc.scalar.activation(
                            sq[:], pg[:], AF.Square,
                            accum_out=qcol(bt, k),
                        )
                if ndve:
                    sv = st.rearrange("p n (a b) -> p n a b", b=3)
                    means = sv[:, :ndve, :, 1]
                    m2s = sv[:, :ndve, :, 2]
                    tmp = stpool.tile([128, NDVE, 2], F32, tag="tmp")
                    peng = nc.gpsimd if GPOFF else nc.vector
                    peng.tensor_tensor(
                        tmp[:, :ndve, :], means, means, op=ALU.mult
                    )
                    nc.vector.scalar_tensor_tensor(
                        tmp[:, :ndve, :], tmp[:, :ndve, :], 256.0, m2s,
                        ALU.mult, ALU.add,
                    )
                    nc.vector.tensor_reduce(
                        qdest(sg, 0, ndve), tmp[:, :ndve, :],
                        axis=mybir.AxisListType.X, op=ALU.add,
                    )
                for bt, k in sg:
                    cols_done[bt] += 1
                while si >= LAG and aff_done < min(
                    n_bt, (si - LAG + 1) * AFF_RATE
                ):
                    emit_affine(aff_done)
                    aff_done += 1
                if aff_done == n_bt:
                    while done_upto < n_bt and cols_done[done_upto] == KSH:
                        emit_assembly(done_upto)
                        done_upto += 1
            while aff_done < n_bt:
                emit_affine(aff_done)
                aff_done += 1
            for bt in range(done_upto, n_bt):
                emit_assembly(bt)
    nc.compile()
    return nc


def prep_inputs(features, A, b):
    """Host-side: chol factors, affine coeffs, fp8 packing, 8 K-shards."""
    np8 = mybir.dt.np(FP8)
    bf = ml_dtypes.bfloat16

    fT = np.ascontiguousarray(features.T)                      # [512, 4096]
    # ft[p, pr, i, b] = fT[(2 pr + i)*128 + p, b]
    ft_host = np.ascontiguousarray(
        fT.reshape(2, 2, 128, B).transpose(2, 0, 1, 3)
    ).astype(np8)

    Ap = np.zeros((KPAD, D, D), dtype=np.float64)
    Ap[:K] = A.astype(np.float64)
    bp = np.zeros((KPAD, D), dtype=np.float64)
    bp[:K] = b.astype(np.float64)

    M = np.einsum('ked,kef->kdf', Ap, Ap)                      # A^T A
    jit = 1e-9 * np.maximum(np.trace(M, axis1=1, axis2=2) / D, 1e-300)
    M += jit[:, None, None] * np.eye(D)[None]
    L = np.linalg.cholesky(M)                                  # [KPAD, D, D]
    Ls = (L * SL).astype(np.float32)

    c2 = 2.0 * np.einsum('ked,ke->kd', Ap, bp) * (2.0 ** SC2_LOG2)
    g = np.sum(bp * bp, axis=1) * (2.0 ** SL2_LOG2)            # [KPAD]

    in_maps = []
    for ci in range(NCORES):
        sl = slice(ci * KSH, (ci + 1) * KSH)
        Lc = Ls[sl]                                            # [13, d, e]
        lt_host = np.zeros((KSH, 128, 2, 768), dtype=np8)
        # full block rows 256..511: lt[k, p, i, e] = L[k, 256+128 i + p, e]
        lt_host[:, :, :, :512] = (
            Lc[:, 256:, :].reshape(KSH, 2, 128, 512).transpose(0, 2, 1, 3)
        ).astype(np8)
        # quarter rows 0..255, cols 0..256
        lt_host[:, :, :, 512:768] = (
            Lc[:, :256, :256].reshape(KSH, 2, 128, 256).transpose(0, 2, 1, 3)
        ).astype(np8)

        c2T = c2[sl].T.astype(np.float32)                      # [512, 13]
        c2_host = np.ascontiguousarray(
            c2T.reshape(2, 2, 128, KSH).transpose(2, 0, 1, 3)
        ).astype(np8)
        gt_host = np.ascontiguousarray(
            np.repeat(g[sl].astype(np.float32)[None, :], 128, axis=0)
        )
        in_maps.append({
            "ftd": ft_host.reshape(128, 16384),
            "ltd": lt_host.reshape(KSH, 128, 1536),
            "c2d": c2_host.reshape(128, 2 * 2 * KSH),
            "gtd": gt_host,
        })
    return in_maps


def _install_ntff_hook():
    """Register the axon NTFF profile hook (missing antenv.axon_hooks shim)."""
    import types
    try:
        import antenv.axon_hooks  # noqa: F401
        return True
    except ImportError:
        pass
    try:
        sys.path.insert(0, "/root/.axon_site")
        from trn_agent_boot.trn_boot import _ntff_profile_via_ctypes
        hook = _ntff_profile_via_ctypes("/opt/axon/libaxon_pjrt.so")
        if hook is None:
            return False
        import antenv
        mod = types.ModuleType("antenv.axon_hooks")
        mod._hook = hook
        mod.get_axon_ntff_profile_hook = lambda: mod._hook
        mod.set_axon_ntff_profile_hook = lambda h: setattr(mod, "_hook", h)
        sys.modules["antenv.axon_hooks"] = mod
        antenv.axon_hooks = mod
        return True
    except Exception as e:  # pragma: no cover
        print(f"ntff hook install failed: {e}", file=sys.stderr)
        return False


def kernel(features: np.ndarray, A: np.ndarray, b: np.ndarray) -> np.ndarray:
    global LAST_EXEC_TIME_NS, LAST_RESULTS
    trace = bool(os.environ.get("BASS_KERNEL_TRACE"))
    kwargs = {}
    if trace:
        if _install_ntff_hook():
            import concourse.bass_utils as bu
            bu.upload_artifacts = lambda tmpdir: f"local:{tmpdir}"
            tmpdir = os.environ.get("BASS_KERNEL_TRACE_DIR") or None
            if tmpdir:
                import glob as _glob
                for f in _glob.glob(os.path.join(tmpdir, "*")):
                    try:
                        os.remove(f)
                    except OSError:
                        pass
            kwargs = dict(trace=True, tmpdir=tmpdir)
        else:
            print("trace requested but NTFF hook unavailable", file=sys.stderr)

    nc = build_nc(NBT)
    in_maps = prep_inputs(
        np.asarray(features, dtype=np.float32),
        np.asarray(A, dtype=np.float32),
        np.asarray(b, dtype=np.float32),
    )
    res = run_bass_kernel_spmd(nc, in_maps, list(range(NCORES)), **kwargs)
    LAST_RESULTS = res
    LAST_EXEC_TIME_NS = res.exec_time_ns
    full = np.concatenate(
        [res.results[i]["dist"] for i in range(NCORES)], axis=1
    )
    return np.ascontiguousarray(full[:, :K]).astype(np.float32)
